# revision 22
# baseline (speedup 1.0000x reference)
"""Multi-head attention (B=4, T=2048, D=1024, H=16, causal) on 8 TRN2 NeuronCores.

Sharding: tensor-parallel over heads — core c owns heads {2c, 2c+1}
(columns [128c, 128c+128) of the QKV projections, rows [128c, 128c+128) of Wo).
Each core computes q/k/v for its heads over all B*T tokens, causal attention,
and a partial output projection; the host sums the 8 partials and adds bo.

Layout/schedule (v8):
- fp16 operands everywhere (PSUM accumulation stays f32); error budget is
  2e-2 and fp16 keeps us ~7e-4.
- "feature-major" activations ([feature, token]) so every matmul contracts
  over the partition dim; scores are computed transposed (S_T[tk, tq]) so the
  softmax needs no transpose before P@V.
- V_aug per tk-tile per head is [64 ones | 64 V] (token-major, via batched
  DMA-XBAR transposes): the PV lhsT is exactly that 128-column window, so PSUM
  rows 0:64 of the PV accumulation are the softmax denominator (at partition
  0, where gpsimd partition_broadcast can read it) and rows 64:128 the head
  output.  NumWeights==128 also enables the compiler's fast weight load.
- Scores/exp/PV are column-restricted to the causal region (c0 = 128*d).
- 1024-wide tq windows, heads sequential; per batch b the attention j-loop is
  woven (emission order = engine order) with the projections of batch b+1 and
  the output projection of batch b-1 / b, keeping the Tensor engine dense so
  the HAM power manager keeps the PE clock up.
- PSUM: proj accumulator [128,1024] (2 banks) + s_pr 2x[128,1024] (4) +
  o_ps [128,1024] (2) = 8 banks.
"""
import sys

sys.path.insert(0, "/opt/trn_rl_repo")

import numpy as np

import concourse.bacc as bacc
import concourse.tile as tile
from concourse import mybir
from concourse.bass_utils import run_bass_kernel_spmd

B, T, D, H, HD = 4, 2048, 1024, 16, 64
NCORES = 8
DPC = 128          # dout per core = 2 heads * 64
BT = B * T         # 8192
TW = 1024          # tq window width
NKT = D // 128     # 8 contraction tiles for projections
NWIN = T // TW     # 2 tq windows per batch
NJW = TW // 128    # 8 tk tiles per window span
HSTRIDE = 128           # per-head V_aug columns: 64 ones then 64 V
VSTRIDE = 2 * HSTRIDE   # 256: per-tk-tile V_aug columns
VONES = 64              # ones block width
VAUGW = (BT // 128) * VSTRIDE
SCALE = 1.0 / np.sqrt(HD)

f16 = mybir.dt.float16
f32 = mybir.dt.float32
MULT = mybir.AluOpType.mult

_cache = {}


def _build(with_bias: bool, debug: bool = False):
    nc = bacc.Bacc()
    xT = nc.dram_tensor("xT", [D, BT], f16, kind="ExternalInput")
    wq = nc.dram_tensor("wq", [D, DPC], f16, kind="ExternalInput")
    wk = nc.dram_tensor("wk", [D, DPC], f16, kind="ExternalInput")
    wv = nc.dram_tensor("wv", [D, DPC], f16, kind="ExternalInput")
    wo = nc.dram_tensor("wo", [DPC, D], f16, kind="ExternalInput")
    out = nc.dram_tensor("out", [D, BT], f16, kind="ExternalOutput")
    if debug:
        dbg_qT = nc.dram_tensor("dbg_qT", [128, BT], f32, kind="ExternalOutput")
        dbg_kT = nc.dram_tensor("dbg_kT", [128, BT], f32, kind="ExternalOutput")
        dbg_va = nc.dram_tensor("dbg_va", [128, VAUGW], f32, kind="ExternalOutput")
        dbg_oT = nc.dram_tensor("dbg_oT", [128, BT], f32, kind="ExternalOutput")
    if with_bias:
        bq = nc.dram_tensor("bq", [DPC, 1], f32, kind="ExternalInput")
        bk = nc.dram_tensor("bk", [DPC, 1], f32, kind="ExternalInput")
        bv = nc.dram_tensor("bv", [DPC, 1], f32, kind="ExternalInput")

    # tri[p, f] = 1.0 if f >= p else 0.0 (keep iff tq >= tk on the diagonal block)
    tri_np = np.zeros((128, 128), dtype=np.float16)
    p_idx = np.arange(128)[:, None]
    f_idx = np.arange(128)[None, :]
    tri_np[f_idx >= p_idx] = 1.0
    tri_dram = nc.inline_tensor(tri_np, name="tri")

    with tile.TileContext(nc) as tc:
        with (
            tc.tile_pool(name="pers", bufs=1) as pers,
            tc.tile_pool(name="xp", bufs=1) as xp,
            tc.tile_pool(name="vs", bufs=2) as vsp,
            tc.tile_pool(name="pp", bufs=1) as ppool,
            tc.tile_pool(name="nrm", bufs=1) as nrm,
            tc.tile_pool(name="outp", bufs=2) as outp,
            tc.tile_pool(name="ps", bufs=1, space="PSUM") as ps,
        ):
            wq_sb = pers.tile([128, D], f16, tag="wq")
            wk_sb = pers.tile([128, D], f16, tag="wk")
            wv_sb = pers.tile([128, D], f16, tag="wv")
            wo_sb = pers.tile([128, D], f16, tag="wo")
            qT = pers.tile([128, BT], f16, tag="qT")
            kT = pers.tile([128, BT], f16, tag="kT")
            oT = pers.tile([128, BT], f16, tag="oT")
            vaug = pers.tile([128, VAUGW], f16, tag="vaug")
            tri_sb = pers.tile([128, 128], f16, tag="tri")

            nc.sync.dma_start(tri_sb[:], tri_dram[:])
            vaug4 = vaug[:].rearrange("p (t g w) -> p t g w", t=BT // 128, g=2)
            nc.gpsimd.memset(vaug4[:, :, :, 0:VONES], 1.0)
            if with_bias:
                bq_sb = pers.tile([128, 1], f32, tag="bq")
                bk_sb = pers.tile([128, 1], f32, tag="bk")
                bv_sb = pers.tile([128, 1], f32, tag="bv")
                nc.sync.dma_start(bq_sb[:], bq[:, :])
                nc.sync.dma_start(bk_sb[:], bk[:, :])
                nc.sync.dma_start(bv_sb[:], bv[:, :])
            # weight loads interleaved with batch-0 x loads so the first proj
            # matmuls (kt=0) have their operands as early as possible
            # one strided DMA per weight matrix: wq_sb[p, kt*128+c] = wq[kt*128+p, c]
            for w_sb, w_dr in ((wq_sb, wq), (wk_sb, wk), (wv_sb, wv)):
                nc.gpsimd.dma_start(
                    w_sb[:].rearrange("p (k c) -> p k c", k=NKT),
                    w_dr[:].rearrange("(k p) c -> p k c", k=NKT),
                )
            xts0 = []
            for kt in range(NKT):
                s = slice(kt * 128, kt * 128 + 128)
                x_t = xp.tile([128, 2048], f16, tag=f"x{kt}", name="x", bufs=2)
                nc.sync.dma_start(x_t[:], xT[s, 0:2048])
                xts0.append(x_t)
            nc.sync.dma_start(wo_sb[:], wo[:, :])

            # ---- stream generators; each yield = one weave step ----

            def proj_stream(b, xts=None):
                """QKV projections for batch b's 2048 tokens (fp16, feature-major).
                One rotating [128,1024] PSUM accumulator; q, k, v sequentially
                per 1024-token supergroup; all x tiles prefetched in a burst."""
                if xts is None:
                    xts = []
                    for kt in range(NKT):
                        s = slice(kt * 128, kt * 128 + 128)
                        x_t = xp.tile([128, 2048], f16, tag=f"x{kt}", name="x", bufs=2)
                        nc.sync.dma_start(x_t[:], xT[s, b * 2048 : b * 2048 + 2048])
                        xts.append(x_t)
                    yield
                for gp2 in (0, 1):
                    gp = 2 * b + gp2
                    for w_sb, wkind in ((wq_sb, "q"), (wk_sb, "k"), (wv_sb, "v")):
                        v_st = None
                        if wkind == "v":
                            v_st = vsp.tile([128, 1024], f16, tag="vst", bufs=2)
                        for half in (0, 1):
                            tok = slice(
                                gp * 1024 + half * 512, gp * 1024 + half * 512 + 512
                            )
                            acc = ps.tile([128, 512], f32, tag="pj", name="pj", bufs=2)
                            for kt in range(NKT):
                                s = slice(kt * 128, kt * 128 + 128)
                                xo = gp2 * 1024 + half * 512
                                nc.tensor.matmul(
                                    acc[:],
                                    w_sb[:, s],
                                    xts[kt][:, xo : xo + 512],
                                    start=kt == 0, stop=kt == NKT - 1,
                                )
                                if kt % 2 == 1:
                                    yield
                            if wkind == "q":
                                if with_bias:
                                    nc.scalar.add(qT[:, tok], acc[:], bq_sb[:])
                                else:
                                    nc.scalar.copy(qT[:, tok], acc[:])
                            elif wkind == "k":
                                if with_bias:
                                    nc.vector.tensor_scalar_add(
                                        kT[:, tok], acc[:], bk_sb[:]
                                    )
                                else:
                                    nc.vector.tensor_copy(kT[:, tok], acc[:])
                            else:
                                hv = v_st[:, half * 512 : half * 512 + 512]
                                if with_bias:
                                    nc.vector.tensor_scalar_add(hv, acc[:], bv_sb[:])
                                else:
                                    nc.vector.tensor_copy(hv, acc[:])
                            yield
                        if wkind == "v":
                            t0 = gp * 8  # first of 8 global tk tile indices
                            for hh in (0, 1):
                                nc.sync.dma_start(
                                    vaug4[:, t0 : t0 + 8, hh, VONES:HSTRIDE],
                                    v_st[hh * HD : hh * HD + HD, :],
                                    transpose=True,
                                )

            def attn_stream(b):
                """Causal attention for batch b, head-sequential per window."""
                tb = b * T
                for wi in range(NWIN):
                    win = slice(tb + wi * TW, tb + wi * TW + TW)
                    jmax = NJW * (wi + 1)
                    for h in (0, 1):
                        hs = slice(h * HD, h * HD + HD)
                        o_ps = ps.tile([128, TW], f32, tag="o", name="o", bufs=1)
                        for j in range(jmax):
                            d = j - NJW * wi
                            c0 = 128 * d if d > 0 else 0
                            bj = slice(tb + j * 128, tb + j * 128 + 128)
                            s_pr = ps.tile([128, TW], f32, tag="s", name="s", bufs=2)
                            for cl, cr in ((c0, 512), (max(c0, 512), TW)):
                                if cl >= cr:
                                    continue
                                nc.tensor.matmul(
                                    s_pr[:, cl:cr],
                                    kT[hs, bj],
                                    qT[hs, win][:, cl:cr],
                                    start=True,
                                    stop=True,
                                )
                            p_pr = ppool.tile([128, TW], f16, tag="p", bufs=6)
                            nc.scalar.activation(
                                p_pr[:, c0:TW],
                                s_pr[:, c0:TW],
                                mybir.ActivationFunctionType.Exp,
                                scale=float(SCALE),
                            )
                            if d >= 0:  # diagonal tile: zero strict lower triangle
                                nc.vector.tensor_tensor(
                                    p_pr[:, c0 : c0 + 128],
                                    p_pr[:, c0 : c0 + 128],
                                    tri_sb[:],
                                    MULT,
                                )
                            vcol = ((tb // 128) + j) * VSTRIDE + h * HSTRIDE
                            for cl, cr in ((c0, 512), (max(c0, 512), TW)):
                                if cl >= cr:
                                    continue
                                nc.tensor.matmul(
                                    o_ps[:, cl:cr],
                                    vaug[:, vcol : vcol + HSTRIDE],
                                    p_pr[:, cl:cr],
                                    start=(j == 0),
                                    stop=(j == jmax - 1) and cr == TW,
                                )
                            yield
                        # normalize: o_ps rows 0:64 are the denominator (ones
                        # block of V_aug), rows 64:128 the head output.
                        o_st = nrm.tile([HSTRIDE, TW], f32, tag="ost", bufs=3)
                        nc.vector.tensor_copy(o_st[:], o_ps[:])
                        bc = nrm.tile([HSTRIDE, TW], f32, tag="bc", bufs=2)
                        nc.gpsimd.partition_broadcast(bc[:], o_st[0:1, :])
                        rc = nrm.tile([HSTRIDE, TW], f32, tag="rc", bufs=2)
                        # full-tile recip at partition base 0 (the custom-DVE
                        # ucode misbehaves on nonzero partition bases)
                        nc.vector.reciprocal_approx_fast(out=rc[:], in_=bc[:])
                        nc.vector.tensor_tensor(
                            oT[hs, win], o_st[VONES:HSTRIDE, :], rc[VONES:HSTRIDE, :], MULT
                        )
                        yield

            def outproj_stream(b, tps=(0, 1)):
                """Partial output projection for batch b: out[:, b] = Wo^T oT.
                outproj for token half tp only needs oT window wi=tp."""
                tb = b * T
                for tp in tps:
                    for dm in range(NKT):
                        s = slice(dm * 128, dm * 128 + 128)
                        o4 = tb + tp * 1024
                        st2 = outp.tile([128, 1024], f16, tag=f"st{dm % 2}", bufs=2)
                        for i2 in range(2):
                            pr = ps.tile([128, 512], f32, tag="pj", name="pr", bufs=2)
                            nc.tensor.matmul(
                                pr[:],
                                wo_sb[:, s],
                                oT[:, o4 + i2 * 512 : o4 + i2 * 512 + 512],
                                start=True, stop=True,
                            )
                            dst = st2[:, i2 * 512 : i2 * 512 + 512]
                            if dm % 2 == 0:
                                nc.scalar.copy(dst, pr[:])
                            else:
                                nc.vector.tensor_copy(dst, pr[:])
                        nc.gpsimd.dma_start(out[s, o4 : o4 + 1024], st2[:])
                        yield

            def drain(stream):
                if stream is not None:
                    for _ in stream:
                        pass

            def adv(st):
                try:
                    next(st)
                    return True
                except StopIteration:
                    return False

            def weave(main, n_main, auxes):
                """auxes: list of [stream, n_steps, start_i, cap_in_loop]."""
                done = [0] * len(auxes)
                for i, _ in enumerate(main, start=1):
                    for a, (st, n, i0, cap) in enumerate(auxes):
                        if st is None or i <= i0:
                            continue
                        span = max(1, n_main - i0)
                        want = min(cap, ((i - i0) * n + span - 1) // span)
                        while done[a] < want and adv(st):
                            done[a] += 1
                for st, _, _, _ in auxes:
                    drain(st)

            # attn steps per batch: sum over wi of 2*(jmax+1)
            N_ATT = sum(2 * (NJW * (wi + 1) + 1) for wi in range(NWIN))  # 52
            N_PROJ = 1 + 2 * 3 * 2 * (NKT // 2 + 1)  # 61
            N_OP = 16

            # ---- schedule ----
            drain(proj_stream(0, xts0))
            for b in range(B):
                auxes = []
                if b + 1 < B:
                    auxes.append((proj_stream(b + 1), N_PROJ, 0, N_PROJ))
                if b - 1 >= 0:
                    auxes.append((outproj_stream(b - 1), N_OP, 0, N_OP))
                if b == B - 1:
                    # gated: tp0 of this batch's outproj after window 0 done
                    auxes.append((outproj_stream(b, (0,)), 8, 20, 8))
                weave(attn_stream(b), N_ATT, auxes)
            drain(outproj_stream(B - 1, (1,)))

            if debug:
                with tc.tile_pool(name="dbgp", bufs=2) as dbgp:
                    for tg in range(BT // 512):
                        tok = slice(tg * 512, tg * 512 + 512)
                        for name, sbuf, dram in (
                            ("q", qT, dbg_qT), ("k", kT, dbg_kT), ("o", oT, dbg_oT)
                        ):
                            t = dbgp.tile([128, 512], f32, tag="d", name="d")
                            nc.vector.tensor_copy(t[:], sbuf[:, tok])
                            nc.sync.dma_start(dram[:, tok], t[:])
                    for c0 in range(0, VAUGW, 512):
                        w = min(512, VAUGW - c0)
                        t = dbgp.tile([128, 512], f32, tag="d", name="d")
                        nc.vector.tensor_copy(t[:, 0:w], vaug[:, c0 : c0 + w])
                        nc.sync.dma_start(dbg_va[:, c0 : c0 + w], t[:, 0:w])

    nc.compile()
    return nc


def _get_nc(with_bias: bool, debug: bool = False):
    key = (with_bias, debug)
    if key not in _cache:
        _cache[key] = _build(with_bias, debug)
    return _cache[key]


def _make_in_maps(x, Wq, bq, Wk, bk, Wv, bv, Wo, with_bias):
    xT = np.ascontiguousarray(x.reshape(BT, D).T.astype(np.float16))
    in_maps = []
    for c in range(NCORES):
        cs = slice(c * DPC, c * DPC + DPC)
        m = {
            "xT": xT,
            "wq": np.ascontiguousarray(Wq[:, cs].astype(np.float16)),
            "wk": np.ascontiguousarray(Wk[:, cs].astype(np.float16)),
            "wv": np.ascontiguousarray(Wv[:, cs].astype(np.float16)),
            "wo": np.ascontiguousarray(Wo[cs, :].astype(np.float16)),
        }
        if with_bias:
            m["bq"] = np.ascontiguousarray(bq[cs]).reshape(DPC, 1).astype(np.float32)
            m["bk"] = np.ascontiguousarray(bk[cs]).reshape(DPC, 1).astype(np.float32)
            m["bv"] = np.ascontiguousarray(bv[cs]).reshape(DPC, 1).astype(np.float32)
        in_maps.append(m)
    return in_maps


def _gather(res, bo):
    acc = np.zeros((D, BT), dtype=np.float32)
    for r in res.results:
        acc += r["out"].astype(np.float32)
    y = acc.T + bo[None, :]
    return np.ascontiguousarray(y.reshape(B, T, D), dtype=np.float32)


def kernel(x, Wq, bq, Wk, bk, Wv, bv, Wo, bo, _trace=False, _debug=False):
    x = np.asarray(x, dtype=np.float32)
    Wq, Wk, Wv, Wo = (np.asarray(w, dtype=np.float32) for w in (Wq, Wk, Wv, Wo))
    bq, bk, bv, bo = (np.asarray(b_, dtype=np.float32) for b_ in (bq, bk, bv, bo))

    with_bias = bool(np.any(bq != 0) or np.any(bk != 0) or np.any(bv != 0))
    nc = _get_nc(with_bias, _debug)
    in_maps = _make_in_maps(x, Wq, bq, Wk, bk, Wv, bv, Wo, with_bias)
    res = run_bass_kernel_spmd(
        nc, in_maps, core_ids=list(range(NCORES)), trace=_trace
    )
    y = _gather(res, bo)
    if _trace or _debug:
        return y, res
    return y


# revision 23
# speedup vs baseline: 1.0022x; 1.0022x over previous
"""Multi-head attention (B=4, T=2048, D=1024, H=16, causal) on 8 TRN2 NeuronCores.

Sharding: tensor-parallel over heads — core c owns heads {2c, 2c+1}
(columns [128c, 128c+128) of the QKV projections, rows [128c, 128c+128) of Wo).
Each core computes q/k/v for its heads over all B*T tokens, causal attention,
and a partial output projection; the host sums the 8 partials and adds bo.

Layout/schedule (v8):
- fp16 operands everywhere (PSUM accumulation stays f32); error budget is
  2e-2 and fp16 keeps us ~7e-4.
- "feature-major" activations ([feature, token]) so every matmul contracts
  over the partition dim; scores are computed transposed (S_T[tk, tq]) so the
  softmax needs no transpose before P@V.
- V_aug per tk-tile per head is [64 ones | 64 V] (token-major, via batched
  DMA-XBAR transposes): the PV lhsT is exactly that 128-column window, so PSUM
  rows 0:64 of the PV accumulation are the softmax denominator (at partition
  0, where gpsimd partition_broadcast can read it) and rows 64:128 the head
  output.  NumWeights==128 also enables the compiler's fast weight load.
- Scores/exp/PV are column-restricted to the causal region (c0 = 128*d).
- 1024-wide tq windows, heads sequential; per batch b the attention j-loop is
  woven (emission order = engine order) with the projections of batch b+1 and
  the output projection of batch b-1 / b, keeping the Tensor engine dense so
  the HAM power manager keeps the PE clock up.
- PSUM: proj accumulator [128,1024] (2 banks) + s_pr 2x[128,1024] (4) +
  o_ps [128,1024] (2) = 8 banks.
"""
import sys

sys.path.insert(0, "/opt/trn_rl_repo")

import numpy as np

import concourse.bacc as bacc
import concourse.tile as tile
from concourse import mybir
from concourse.bass_utils import run_bass_kernel_spmd

B, T, D, H, HD = 4, 2048, 1024, 16, 64
NCORES = 8
DPC = 128          # dout per core = 2 heads * 64
BT = B * T         # 8192
TW = 1024          # tq window width
NKT = D // 128     # 8 contraction tiles for projections
NWIN = T // TW     # 2 tq windows per batch
NJW = TW // 128    # 8 tk tiles per window span
HSTRIDE = 128           # per-head V_aug columns: 64 ones then 64 V
VSTRIDE = 2 * HSTRIDE   # 256: per-tk-tile V_aug columns
VONES = 64              # ones block width
VAUGW = (BT // 128) * VSTRIDE
SCALE = 1.0 / np.sqrt(HD)

f16 = mybir.dt.float16
f32 = mybir.dt.float32
MULT = mybir.AluOpType.mult

_cache = {}


def _build(with_bias: bool, debug: bool = False):
    nc = bacc.Bacc()
    xT = nc.dram_tensor("xT", [D, BT], f16, kind="ExternalInput")
    wq = nc.dram_tensor("wq", [D, DPC], f16, kind="ExternalInput")
    wk = nc.dram_tensor("wk", [D, DPC], f16, kind="ExternalInput")
    wv = nc.dram_tensor("wv", [D, DPC], f16, kind="ExternalInput")
    wo = nc.dram_tensor("wo", [DPC, D], f16, kind="ExternalInput")
    out = nc.dram_tensor("out", [D, BT], f16, kind="ExternalOutput")
    if debug:
        dbg_qT = nc.dram_tensor("dbg_qT", [128, BT], f32, kind="ExternalOutput")
        dbg_kT = nc.dram_tensor("dbg_kT", [128, BT], f32, kind="ExternalOutput")
        dbg_va = nc.dram_tensor("dbg_va", [128, VAUGW], f32, kind="ExternalOutput")
        dbg_oT = nc.dram_tensor("dbg_oT", [128, BT], f32, kind="ExternalOutput")
    if with_bias:
        bq = nc.dram_tensor("bq", [DPC, 1], f32, kind="ExternalInput")
        bk = nc.dram_tensor("bk", [DPC, 1], f32, kind="ExternalInput")
        bv = nc.dram_tensor("bv", [DPC, 1], f32, kind="ExternalInput")

    # tri[p, f] = 1.0 if f >= p else 0.0 (keep iff tq >= tk on the diagonal block)
    tri_np = np.zeros((128, 128), dtype=np.float16)
    p_idx = np.arange(128)[:, None]
    f_idx = np.arange(128)[None, :]
    tri_np[f_idx >= p_idx] = 1.0
    tri_dram = nc.inline_tensor(tri_np, name="tri")

    with tile.TileContext(nc) as tc:
        with (
            tc.tile_pool(name="pers", bufs=1) as pers,
            tc.tile_pool(name="xp", bufs=1) as xp,
            tc.tile_pool(name="vs", bufs=2) as vsp,
            tc.tile_pool(name="pp", bufs=1) as ppool,
            tc.tile_pool(name="nrm", bufs=1) as nrm,
            tc.tile_pool(name="outp", bufs=2) as outp,
            tc.tile_pool(name="ps", bufs=1, space="PSUM") as ps,
        ):
            wq_sb = pers.tile([128, D], f16, tag="wq")
            wk_sb = pers.tile([128, D], f16, tag="wk")
            wv_sb = pers.tile([128, D], f16, tag="wv")
            wo_sb = pers.tile([128, D], f16, tag="wo")
            qT = pers.tile([128, BT], f16, tag="qT")
            kT = pers.tile([128, BT], f16, tag="kT")
            oT = pers.tile([128, BT], f16, tag="oT")
            vaug = pers.tile([128, VAUGW], f16, tag="vaug")
            tri_sb = pers.tile([128, 128], f16, tag="tri")

            nc.sync.dma_start(tri_sb[:], tri_dram[:])
            vaug4 = vaug[:].rearrange("p (t g w) -> p t g w", t=BT // 128, g=2)
            nc.gpsimd.memset(vaug4[:, :, :, 0:VONES], 1.0)
            if with_bias:
                bq_sb = pers.tile([128, 1], f32, tag="bq")
                bk_sb = pers.tile([128, 1], f32, tag="bk")
                bv_sb = pers.tile([128, 1], f32, tag="bv")
                nc.sync.dma_start(bq_sb[:], bq[:, :])
                nc.sync.dma_start(bk_sb[:], bk[:, :])
                nc.sync.dma_start(bv_sb[:], bv[:, :])
            # weight loads interleaved with batch-0 x loads so the first proj
            # matmuls (kt=0) have their operands as early as possible
            # one strided DMA per weight matrix: wq_sb[p, kt*128+c] = wq[kt*128+p, c]
            for w_sb, w_dr in ((wq_sb, wq), (wk_sb, wk), (wv_sb, wv)):
                nc.gpsimd.dma_start(
                    w_sb[:].rearrange("p (k c) -> p k c", k=NKT),
                    w_dr[:].rearrange("(k p) c -> p k c", k=NKT),
                )
            xts0 = []
            for kt in range(NKT):
                s = slice(kt * 128, kt * 128 + 128)
                x_t = xp.tile([128, 2048], f16, tag=f"x{kt}", name="x", bufs=2)
                nc.sync.dma_start(x_t[:], xT[s, 0:2048])
                xts0.append(x_t)
            nc.sync.dma_start(wo_sb[:], wo[:, :])

            # ---- stream generators; each yield = one weave step ----

            def proj_stream(b, xts=None):
                """QKV projections for batch b's 2048 tokens (fp16, feature-major).
                One rotating [128,1024] PSUM accumulator; q, k, v sequentially
                per 1024-token supergroup; all x tiles prefetched in a burst."""
                if xts is None:
                    xts = []
                    for kt in range(NKT):
                        s = slice(kt * 128, kt * 128 + 128)
                        x_t = xp.tile([128, 2048], f16, tag=f"x{kt}", name="x", bufs=2)
                        nc.sync.dma_start(x_t[:], xT[s, b * 2048 : b * 2048 + 2048])
                        xts.append(x_t)
                    yield
                for gp2 in (0, 1):
                    gp = 2 * b + gp2
                    for w_sb, wkind in ((wq_sb, "q"), (wk_sb, "k"), (wv_sb, "v")):
                        v_st = None
                        if wkind == "v":
                            v_st = vsp.tile([128, 1024], f16, tag="vst", bufs=2)
                        for half in (0, 1):
                            tok = slice(
                                gp * 1024 + half * 512, gp * 1024 + half * 512 + 512
                            )
                            acc = ps.tile([128, 512], f32, tag="pj", name="pj", bufs=2)
                            for kt in range(NKT):
                                s = slice(kt * 128, kt * 128 + 128)
                                xo = gp2 * 1024 + half * 512
                                nc.tensor.matmul(
                                    acc[:],
                                    w_sb[:, s],
                                    xts[kt][:, xo : xo + 512],
                                    start=kt == 0, stop=kt == NKT - 1,
                                )
                                if kt % 2 == 1:
                                    yield
                            if wkind == "q":
                                if with_bias:
                                    nc.scalar.add(qT[:, tok], acc[:], bq_sb[:])
                                else:
                                    nc.scalar.copy(qT[:, tok], acc[:])
                            elif wkind == "k":
                                if with_bias:
                                    nc.vector.tensor_scalar_add(
                                        kT[:, tok], acc[:], bk_sb[:]
                                    )
                                else:
                                    nc.vector.tensor_copy(kT[:, tok], acc[:])
                            else:
                                hv = v_st[:, half * 512 : half * 512 + 512]
                                if with_bias:
                                    nc.vector.tensor_scalar_add(hv, acc[:], bv_sb[:])
                                else:
                                    nc.vector.tensor_copy(hv, acc[:])
                            yield
                        if wkind == "v":
                            t0 = gp * 8  # first of 8 global tk tile indices
                            for hh in (0, 1):
                                nc.sync.dma_start(
                                    vaug4[:, t0 : t0 + 8, hh, VONES:HSTRIDE],
                                    v_st[hh * HD : hh * HD + HD, :],
                                    transpose=True,
                                )

            def attn_stream(b):
                """Causal attention for batch b, head-sequential per window."""
                tb = b * T
                for wi in range(NWIN):
                    win = slice(tb + wi * TW, tb + wi * TW + TW)
                    jmax = NJW * (wi + 1)
                    for h in (0, 1):
                        hs = slice(h * HD, h * HD + HD)
                        o_ps = ps.tile([128, TW], f32, tag="o", name="o", bufs=1)
                        for j in range(jmax):
                            d = j - NJW * wi
                            c0 = 128 * d if d > 0 else 0
                            bj = slice(tb + j * 128, tb + j * 128 + 128)
                            s_pr = ps.tile([128, TW], f32, tag="s", name="s", bufs=2)
                            for cl, cr in ((c0, 512), (max(c0, 512), TW)):
                                if cl >= cr:
                                    continue
                                nc.tensor.matmul(
                                    s_pr[:, cl:cr],
                                    kT[hs, bj],
                                    qT[hs, win][:, cl:cr],
                                    start=True,
                                    stop=True,
                                )
                            p_pr = ppool.tile([128, TW], f16, tag="p", bufs=6)
                            nc.scalar.activation(
                                p_pr[:, c0:TW],
                                s_pr[:, c0:TW],
                                mybir.ActivationFunctionType.Exp,
                                scale=float(SCALE),
                            )
                            if d >= 0:  # diagonal tile: zero strict lower triangle
                                nc.vector.tensor_tensor(
                                    p_pr[:, c0 : c0 + 128],
                                    p_pr[:, c0 : c0 + 128],
                                    tri_sb[:],
                                    MULT,
                                )
                            vcol = ((tb // 128) + j) * VSTRIDE + h * HSTRIDE
                            for cl, cr in ((c0, 512), (max(c0, 512), TW)):
                                if cl >= cr:
                                    continue
                                nc.tensor.matmul(
                                    o_ps[:, cl:cr],
                                    vaug[:, vcol : vcol + HSTRIDE],
                                    p_pr[:, cl:cr],
                                    start=(j == 0),
                                    stop=(j == jmax - 1) and cr == TW,
                                )
                            yield
                        # normalize: o_ps rows 0:64 are the denominator (ones
                        # block of V_aug), rows 64:128 the head output.
                        o_st = nrm.tile([HSTRIDE, TW], f32, tag="ost", bufs=3)
                        nc.vector.tensor_copy(o_st[:], o_ps[:])
                        bc = nrm.tile([HSTRIDE, TW], f32, tag="bc", bufs=2)
                        nc.gpsimd.partition_broadcast(bc[:], o_st[0:1, :])
                        rc = nrm.tile([HSTRIDE, TW], f32, tag="rc", bufs=2)
                        # full-tile recip at partition base 0 (the custom-DVE
                        # ucode misbehaves on nonzero partition bases)
                        nc.vector.reciprocal_approx_fast(out=rc[:], in_=bc[:])
                        nc.vector.tensor_tensor(
                            oT[hs, win], o_st[VONES:HSTRIDE, :], rc[VONES:HSTRIDE, :], MULT
                        )
                        yield

            def outproj_stream(b, tps=(0, 1)):
                """Partial output projection for batch b: out[:, b] = Wo^T oT.
                outproj for token half tp only needs oT window wi=tp."""
                tb = b * T
                for tp in tps:
                    for dm in range(NKT):
                        s = slice(dm * 128, dm * 128 + 128)
                        o4 = tb + tp * 1024
                        st2 = outp.tile([128, 1024], f16, tag=f"st{dm % 2}", bufs=2)
                        for i2 in range(2):
                            pr = ps.tile([128, 512], f32, tag="pj", name="pr", bufs=2)
                            nc.tensor.matmul(
                                pr[:],
                                wo_sb[:, s],
                                oT[:, o4 + i2 * 512 : o4 + i2 * 512 + 512],
                                start=True, stop=True,
                            )
                            dst = st2[:, i2 * 512 : i2 * 512 + 512]
                            if dm % 2 == 0:
                                nc.scalar.copy(dst, pr[:])
                            else:
                                nc.vector.tensor_copy(dst, pr[:])
                        nc.sync.dma_start(out[s, o4 : o4 + 1024], st2[:])
                        yield

            def drain(stream):
                if stream is not None:
                    for _ in stream:
                        pass

            def adv(st):
                try:
                    next(st)
                    return True
                except StopIteration:
                    return False

            def weave(main, n_main, auxes):
                """auxes: list of [stream, n_steps, start_i, cap_in_loop]."""
                done = [0] * len(auxes)
                for i, _ in enumerate(main, start=1):
                    for a, (st, n, i0, cap) in enumerate(auxes):
                        if st is None or i <= i0:
                            continue
                        span = max(1, n_main - i0)
                        want = min(cap, ((i - i0) * n + span - 1) // span)
                        while done[a] < want and adv(st):
                            done[a] += 1
                for st, _, _, _ in auxes:
                    drain(st)

            # attn steps per batch: sum over wi of 2*(jmax+1)
            N_ATT = sum(2 * (NJW * (wi + 1) + 1) for wi in range(NWIN))  # 52
            N_PROJ = 1 + 2 * 3 * 2 * (NKT // 2 + 1)  # 61
            N_OP = 16

            # ---- schedule ----
            drain(proj_stream(0, xts0))
            for b in range(B):
                auxes = []
                if b + 1 < B:
                    auxes.append((proj_stream(b + 1), N_PROJ, 0, N_PROJ))
                if b - 1 >= 0:
                    auxes.append((outproj_stream(b - 1), N_OP, 0, N_OP))
                if b == B - 1:
                    # gated: tp0 of this batch's outproj after window 0 done
                    auxes.append((outproj_stream(b, (0,)), 8, 20, 8))
                weave(attn_stream(b), N_ATT, auxes)
            drain(outproj_stream(B - 1, (1,)))

            if debug:
                with tc.tile_pool(name="dbgp", bufs=2) as dbgp:
                    for tg in range(BT // 512):
                        tok = slice(tg * 512, tg * 512 + 512)
                        for name, sbuf, dram in (
                            ("q", qT, dbg_qT), ("k", kT, dbg_kT), ("o", oT, dbg_oT)
                        ):
                            t = dbgp.tile([128, 512], f32, tag="d", name="d")
                            nc.vector.tensor_copy(t[:], sbuf[:, tok])
                            nc.sync.dma_start(dram[:, tok], t[:])
                    for c0 in range(0, VAUGW, 512):
                        w = min(512, VAUGW - c0)
                        t = dbgp.tile([128, 512], f32, tag="d", name="d")
                        nc.vector.tensor_copy(t[:, 0:w], vaug[:, c0 : c0 + w])
                        nc.sync.dma_start(dbg_va[:, c0 : c0 + w], t[:, 0:w])

    nc.compile()
    return nc


def _get_nc(with_bias: bool, debug: bool = False):
    key = (with_bias, debug)
    if key not in _cache:
        _cache[key] = _build(with_bias, debug)
    return _cache[key]


def _make_in_maps(x, Wq, bq, Wk, bk, Wv, bv, Wo, with_bias):
    xT = np.ascontiguousarray(x.reshape(BT, D).T.astype(np.float16))
    in_maps = []
    for c in range(NCORES):
        cs = slice(c * DPC, c * DPC + DPC)
        m = {
            "xT": xT,
            "wq": np.ascontiguousarray(Wq[:, cs].astype(np.float16)),
            "wk": np.ascontiguousarray(Wk[:, cs].astype(np.float16)),
            "wv": np.ascontiguousarray(Wv[:, cs].astype(np.float16)),
            "wo": np.ascontiguousarray(Wo[cs, :].astype(np.float16)),
        }
        if with_bias:
            m["bq"] = np.ascontiguousarray(bq[cs]).reshape(DPC, 1).astype(np.float32)
            m["bk"] = np.ascontiguousarray(bk[cs]).reshape(DPC, 1).astype(np.float32)
            m["bv"] = np.ascontiguousarray(bv[cs]).reshape(DPC, 1).astype(np.float32)
        in_maps.append(m)
    return in_maps


def _gather(res, bo):
    acc = np.zeros((D, BT), dtype=np.float32)
    for r in res.results:
        acc += r["out"].astype(np.float32)
    y = acc.T + bo[None, :]
    return np.ascontiguousarray(y.reshape(B, T, D), dtype=np.float32)


def kernel(x, Wq, bq, Wk, bk, Wv, bv, Wo, bo, _trace=False, _debug=False):
    x = np.asarray(x, dtype=np.float32)
    Wq, Wk, Wv, Wo = (np.asarray(w, dtype=np.float32) for w in (Wq, Wk, Wv, Wo))
    bq, bk, bv, bo = (np.asarray(b_, dtype=np.float32) for b_ in (bq, bk, bv, bo))

    with_bias = bool(np.any(bq != 0) or np.any(bk != 0) or np.any(bv != 0))
    nc = _get_nc(with_bias, _debug)
    in_maps = _make_in_maps(x, Wq, bq, Wk, bk, Wv, bv, Wo, with_bias)
    res = run_bass_kernel_spmd(
        nc, in_maps, core_ids=list(range(NCORES)), trace=_trace
    )
    y = _gather(res, bo)
    if _trace or _debug:
        return y, res
    return y


# revision 24
# speedup vs baseline: 1.0158x; 1.0136x over previous
"""Multi-head attention (B=4, T=2048, D=1024, H=16, causal) on 8 TRN2 NeuronCores.

Sharding: tensor-parallel over heads — core c owns heads {2c, 2c+1}
(columns [128c, 128c+128) of the QKV projections, rows [128c, 128c+128) of Wo).
Each core computes q/k/v for its heads over all B*T tokens, causal attention,
and a partial output projection; the host sums the 8 partials and adds bo.

Layout/schedule (v8):
- fp16 operands everywhere (PSUM accumulation stays f32); error budget is
  2e-2 and fp16 keeps us ~7e-4.
- "feature-major" activations ([feature, token]) so every matmul contracts
  over the partition dim; scores are computed transposed (S_T[tk, tq]) so the
  softmax needs no transpose before P@V.
- V_aug per tk-tile per head is [64 ones | 64 V] (token-major, via batched
  DMA-XBAR transposes): the PV lhsT is exactly that 128-column window, so PSUM
  rows 0:64 of the PV accumulation are the softmax denominator (at partition
  0, where gpsimd partition_broadcast can read it) and rows 64:128 the head
  output.  NumWeights==128 also enables the compiler's fast weight load.
- Scores/exp/PV are column-restricted to the causal region (c0 = 128*d).
- 1024-wide tq windows, heads sequential; per batch b the attention j-loop is
  woven (emission order = engine order) with the projections of batch b+1 and
  the output projection of batch b-1 / b, keeping the Tensor engine dense so
  the HAM power manager keeps the PE clock up.
- PSUM: proj accumulator [128,1024] (2 banks) + s_pr 2x[128,1024] (4) +
  o_ps [128,1024] (2) = 8 banks.
"""
import sys

sys.path.insert(0, "/opt/trn_rl_repo")

import numpy as np

import concourse.bacc as bacc
import concourse.tile as tile
from concourse import mybir
from concourse.bass_utils import run_bass_kernel_spmd

B, T, D, H, HD = 4, 2048, 1024, 16, 64
NCORES = 8
DPC = 128          # dout per core = 2 heads * 64
BT = B * T         # 8192
TW = 1024          # tq window width
NKT = D // 128     # 8 contraction tiles for projections
NWIN = T // TW     # 2 tq windows per batch
NJW = TW // 128    # 8 tk tiles per window span
HSTRIDE = 128           # per-head V_aug columns: 64 ones then 64 V
VSTRIDE = 2 * HSTRIDE   # 256: per-tk-tile V_aug columns
VONES = 64              # ones block width
VAUGW = (BT // 128) * VSTRIDE
SCALE = 1.0 / np.sqrt(HD)

f16 = mybir.dt.float16
f32 = mybir.dt.float32
MULT = mybir.AluOpType.mult

_cache = {}


def _build(with_bias: bool, debug: bool = False):
    nc = bacc.Bacc()
    xT = nc.dram_tensor("xT", [D, BT], f16, kind="ExternalInput")
    wq = nc.dram_tensor("wq", [D, DPC], f16, kind="ExternalInput")
    wk = nc.dram_tensor("wk", [D, DPC], f16, kind="ExternalInput")
    wv = nc.dram_tensor("wv", [D, DPC], f16, kind="ExternalInput")
    wo = nc.dram_tensor("wo", [DPC, D], f16, kind="ExternalInput")
    out = nc.dram_tensor("out", [D, BT], f16, kind="ExternalOutput")
    if debug:
        dbg_qT = nc.dram_tensor("dbg_qT", [128, BT], f32, kind="ExternalOutput")
        dbg_kT = nc.dram_tensor("dbg_kT", [128, BT], f32, kind="ExternalOutput")
        dbg_va = nc.dram_tensor("dbg_va", [128, VAUGW], f32, kind="ExternalOutput")
        dbg_oT = nc.dram_tensor("dbg_oT", [128, BT], f32, kind="ExternalOutput")
    if with_bias:
        bq = nc.dram_tensor("bq", [DPC, 1], f32, kind="ExternalInput")
        bk = nc.dram_tensor("bk", [DPC, 1], f32, kind="ExternalInput")
        bv = nc.dram_tensor("bv", [DPC, 1], f32, kind="ExternalInput")

    # tri[p, f] = 1.0 if f >= p else 0.0 (keep iff tq >= tk on the diagonal block)
    tri_np = np.zeros((128, 128), dtype=np.float16)
    p_idx = np.arange(128)[:, None]
    f_idx = np.arange(128)[None, :]
    tri_np[f_idx >= p_idx] = 1.0
    tri_dram = nc.inline_tensor(tri_np, name="tri")

    with tile.TileContext(nc) as tc:
        with (
            tc.tile_pool(name="pers", bufs=1) as pers,
            tc.tile_pool(name="xp", bufs=1) as xp,
            tc.tile_pool(name="vs", bufs=2) as vsp,
            tc.tile_pool(name="pp", bufs=1) as ppool,
            tc.tile_pool(name="nrm", bufs=1) as nrm,
            tc.tile_pool(name="outp", bufs=2) as outp,
            tc.tile_pool(name="ps", bufs=1, space="PSUM") as ps,
        ):
            wq_sb = pers.tile([128, D], f16, tag="wq")
            wk_sb = pers.tile([128, D], f16, tag="wk")
            wv_sb = pers.tile([128, D], f16, tag="wv")
            wo_sb = pers.tile([128, D], f16, tag="wo")
            qT = pers.tile([128, BT], f16, tag="qT")
            kT = pers.tile([128, BT], f16, tag="kT")
            oT = pers.tile([128, BT], f16, tag="oT")
            vaug = pers.tile([128, VAUGW], f16, tag="vaug")
            tri_sb = pers.tile([128, 128], f16, tag="tri")

            nc.sync.dma_start(tri_sb[:], tri_dram[:])
            vaug4 = vaug[:].rearrange("p (t g w) -> p t g w", t=BT // 128, g=2)
            nc.gpsimd.memset(vaug4[:, :, :, 0:VONES], 1.0)
            if with_bias:
                bq_sb = pers.tile([128, 1], f32, tag="bq")
                bk_sb = pers.tile([128, 1], f32, tag="bk")
                bv_sb = pers.tile([128, 1], f32, tag="bv")
                nc.sync.dma_start(bq_sb[:], bq[:, :])
                nc.sync.dma_start(bk_sb[:], bk[:, :])
                nc.sync.dma_start(bv_sb[:], bv[:, :])
            # weight loads interleaved with batch-0 x loads so the first proj
            # matmuls (kt=0) have their operands as early as possible
            # one strided DMA per weight matrix: wq_sb[p, kt*128+c] = wq[kt*128+p, c]
            for w_sb, w_dr in ((wq_sb, wq), (wk_sb, wk), (wv_sb, wv)):
                nc.sync.dma_start(
                    w_sb[:].rearrange("p (k c) -> p k c", k=NKT),
                    w_dr[:].rearrange("(k p) c -> p k c", k=NKT),
                )
            xts0 = []
            for kt in range(NKT):
                s = slice(kt * 128, kt * 128 + 128)
                x_t = xp.tile([128, 2048], f16, tag=f"x{kt}", name="x", bufs=2)
                nc.sync.dma_start(x_t[:], xT[s, 0:2048])
                xts0.append(x_t)
            nc.sync.dma_start(wo_sb[:], wo[:, :])

            # ---- stream generators; each yield = one weave step ----

            def proj_stream(b, xts=None):
                """QKV projections for batch b's 2048 tokens (fp16, feature-major).
                One rotating [128,1024] PSUM accumulator; q, k, v sequentially
                per 1024-token supergroup; all x tiles prefetched in a burst."""
                if xts is None:
                    xts = []
                    for kt in range(NKT):
                        s = slice(kt * 128, kt * 128 + 128)
                        x_t = xp.tile([128, 2048], f16, tag=f"x{kt}", name="x", bufs=2)
                        nc.sync.dma_start(x_t[:], xT[s, b * 2048 : b * 2048 + 2048])
                        xts.append(x_t)
                    yield
                for gp2 in (0, 1):
                    gp = 2 * b + gp2
                    for w_sb, wkind in ((wq_sb, "q"), (wk_sb, "k"), (wv_sb, "v")):
                        v_st = None
                        if wkind == "v":
                            v_st = vsp.tile([128, 1024], f16, tag="vst", bufs=2)
                        for half in (0, 1):
                            tok = slice(
                                gp * 1024 + half * 512, gp * 1024 + half * 512 + 512
                            )
                            acc = ps.tile([128, 512], f32, tag="pj", name="pj", bufs=2)
                            for kt in range(NKT):
                                s = slice(kt * 128, kt * 128 + 128)
                                xo = gp2 * 1024 + half * 512
                                nc.tensor.matmul(
                                    acc[:],
                                    w_sb[:, s],
                                    xts[kt][:, xo : xo + 512],
                                    start=kt == 0, stop=kt == NKT - 1,
                                )
                                if kt % 2 == 1:
                                    yield
                            if wkind == "q":
                                if with_bias:
                                    nc.scalar.add(qT[:, tok], acc[:], bq_sb[:])
                                else:
                                    nc.scalar.copy(qT[:, tok], acc[:])
                            elif wkind == "k":
                                if with_bias:
                                    nc.vector.tensor_scalar_add(
                                        kT[:, tok], acc[:], bk_sb[:]
                                    )
                                else:
                                    nc.vector.tensor_copy(kT[:, tok], acc[:])
                            else:
                                hv = v_st[:, half * 512 : half * 512 + 512]
                                if with_bias:
                                    nc.vector.tensor_scalar_add(hv, acc[:], bv_sb[:])
                                else:
                                    nc.vector.tensor_copy(hv, acc[:])
                            yield
                        if wkind == "v":
                            t0 = gp * 8  # first of 8 global tk tile indices
                            for hh in (0, 1):
                                nc.sync.dma_start(
                                    vaug4[:, t0 : t0 + 8, hh, VONES:HSTRIDE],
                                    v_st[hh * HD : hh * HD + HD, :],
                                    transpose=True,
                                )

            def attn_stream(b):
                """Causal attention for batch b, head-sequential per window."""
                tb = b * T
                for wi in range(NWIN):
                    win = slice(tb + wi * TW, tb + wi * TW + TW)
                    jmax = NJW * (wi + 1)
                    for h in (0, 1):
                        hs = slice(h * HD, h * HD + HD)
                        o_ps = ps.tile([128, TW], f32, tag="o", name="o", bufs=1)
                        for j in range(jmax):
                            d = j - NJW * wi
                            c0 = 128 * d if d > 0 else 0
                            bj = slice(tb + j * 128, tb + j * 128 + 128)
                            s_pr = ps.tile([128, TW], f32, tag="s", name="s", bufs=2)
                            for cl, cr in ((c0, 512), (max(c0, 512), TW)):
                                if cl >= cr:
                                    continue
                                nc.tensor.matmul(
                                    s_pr[:, cl:cr],
                                    kT[hs, bj],
                                    qT[hs, win][:, cl:cr],
                                    start=True,
                                    stop=True,
                                )
                            p_pr = ppool.tile([128, TW], f16, tag="p", bufs=6)
                            nc.scalar.activation(
                                p_pr[:, c0:TW],
                                s_pr[:, c0:TW],
                                mybir.ActivationFunctionType.Exp,
                                scale=float(SCALE),
                            )
                            if d >= 0:  # diagonal tile: zero strict lower triangle
                                nc.vector.tensor_tensor(
                                    p_pr[:, c0 : c0 + 128],
                                    p_pr[:, c0 : c0 + 128],
                                    tri_sb[:],
                                    MULT,
                                )
                            vcol = ((tb // 128) + j) * VSTRIDE + h * HSTRIDE
                            for cl, cr in ((c0, 512), (max(c0, 512), TW)):
                                if cl >= cr:
                                    continue
                                nc.tensor.matmul(
                                    o_ps[:, cl:cr],
                                    vaug[:, vcol : vcol + HSTRIDE],
                                    p_pr[:, cl:cr],
                                    start=(j == 0),
                                    stop=(j == jmax - 1) and cr == TW,
                                )
                            yield
                        # normalize: o_ps rows 0:64 are the denominator (ones
                        # block of V_aug), rows 64:128 the head output.
                        o_st = nrm.tile([HSTRIDE, TW], f32, tag="ost", bufs=3)
                        nc.vector.tensor_copy(o_st[:], o_ps[:])
                        bc = nrm.tile([HSTRIDE, TW], f32, tag="bc", bufs=2)
                        nc.gpsimd.partition_broadcast(bc[:], o_st[0:1, :])
                        rc = nrm.tile([HSTRIDE, TW], f32, tag="rc", bufs=2)
                        # full-tile recip at partition base 0 (the custom-DVE
                        # ucode misbehaves on nonzero partition bases)
                        nc.vector.reciprocal_approx_fast(out=rc[:], in_=bc[:])
                        nc.vector.tensor_tensor(
                            oT[hs, win], o_st[VONES:HSTRIDE, :], rc[VONES:HSTRIDE, :], MULT
                        )
                        yield

            def outproj_stream(b, tps=(0, 1)):
                """Partial output projection for batch b: out[:, b] = Wo^T oT.
                outproj for token half tp only needs oT window wi=tp."""
                tb = b * T
                for tp in tps:
                    for dm in range(NKT):
                        s = slice(dm * 128, dm * 128 + 128)
                        o4 = tb + tp * 1024
                        st2 = outp.tile([128, 1024], f16, tag=f"st{dm % 2}", bufs=2)
                        for i2 in range(2):
                            pr = ps.tile([128, 512], f32, tag="pj", name="pr", bufs=2)
                            nc.tensor.matmul(
                                pr[:],
                                wo_sb[:, s],
                                oT[:, o4 + i2 * 512 : o4 + i2 * 512 + 512],
                                start=True, stop=True,
                            )
                            dst = st2[:, i2 * 512 : i2 * 512 + 512]
                            if dm % 2 == 0:
                                nc.scalar.copy(dst, pr[:])
                            else:
                                nc.vector.tensor_copy(dst, pr[:])
                        nc.sync.dma_start(out[s, o4 : o4 + 1024], st2[:])
                        yield

            def drain(stream):
                if stream is not None:
                    for _ in stream:
                        pass

            def adv(st):
                try:
                    next(st)
                    return True
                except StopIteration:
                    return False

            def weave(main, n_main, auxes):
                """auxes: list of [stream, n_steps, start_i, cap_in_loop]."""
                done = [0] * len(auxes)
                for i, _ in enumerate(main, start=1):
                    for a, (st, n, i0, cap) in enumerate(auxes):
                        if st is None or i <= i0:
                            continue
                        want = min(cap, (i - i0) * n // max(1, n_main - i0))
                        while done[a] < want and adv(st):
                            done[a] += 1
                for st, _, _, _ in auxes:
                    drain(st)

            # attn steps per batch: sum over wi of 2*(jmax+1)
            N_ATT = sum(2 * (NJW * (wi + 1) + 1) for wi in range(NWIN))  # 52
            N_PROJ = 1 + 2 * 3 * 2 * (NKT // 2 + 1)  # 61
            N_OP = 16

            # ---- schedule ----
            drain(proj_stream(0, xts0))
            for b in range(B):
                auxes = []
                if b + 1 < B:
                    auxes.append((proj_stream(b + 1), N_PROJ, 0, N_PROJ))
                if b - 1 >= 0:
                    auxes.append((outproj_stream(b - 1), N_OP, 0, N_OP))
                if b == B - 1:
                    # gated: tp0 of this batch's outproj after window 0 done
                    auxes.append((outproj_stream(b, (0,)), 8, 20, 8))
                weave(attn_stream(b), N_ATT, auxes)
            drain(outproj_stream(B - 1, (1,)))

            if debug:
                with tc.tile_pool(name="dbgp", bufs=2) as dbgp:
                    for tg in range(BT // 512):
                        tok = slice(tg * 512, tg * 512 + 512)
                        for name, sbuf, dram in (
                            ("q", qT, dbg_qT), ("k", kT, dbg_kT), ("o", oT, dbg_oT)
                        ):
                            t = dbgp.tile([128, 512], f32, tag="d", name="d")
                            nc.vector.tensor_copy(t[:], sbuf[:, tok])
                            nc.sync.dma_start(dram[:, tok], t[:])
                    for c0 in range(0, VAUGW, 512):
                        w = min(512, VAUGW - c0)
                        t = dbgp.tile([128, 512], f32, tag="d", name="d")
                        nc.vector.tensor_copy(t[:, 0:w], vaug[:, c0 : c0 + w])
                        nc.sync.dma_start(dbg_va[:, c0 : c0 + w], t[:, 0:w])

    nc.compile()
    return nc


def _get_nc(with_bias: bool, debug: bool = False):
    key = (with_bias, debug)
    if key not in _cache:
        _cache[key] = _build(with_bias, debug)
    return _cache[key]


def _make_in_maps(x, Wq, bq, Wk, bk, Wv, bv, Wo, with_bias):
    xT = np.ascontiguousarray(x.reshape(BT, D).T.astype(np.float16))
    in_maps = []
    for c in range(NCORES):
        cs = slice(c * DPC, c * DPC + DPC)
        m = {
            "xT": xT,
            "wq": np.ascontiguousarray(Wq[:, cs].astype(np.float16)),
            "wk": np.ascontiguousarray(Wk[:, cs].astype(np.float16)),
            "wv": np.ascontiguousarray(Wv[:, cs].astype(np.float16)),
            "wo": np.ascontiguousarray(Wo[cs, :].astype(np.float16)),
        }
        if with_bias:
            m["bq"] = np.ascontiguousarray(bq[cs]).reshape(DPC, 1).astype(np.float32)
            m["bk"] = np.ascontiguousarray(bk[cs]).reshape(DPC, 1).astype(np.float32)
            m["bv"] = np.ascontiguousarray(bv[cs]).reshape(DPC, 1).astype(np.float32)
        in_maps.append(m)
    return in_maps


def _gather(res, bo):
    acc = np.zeros((D, BT), dtype=np.float32)
    for r in res.results:
        acc += r["out"].astype(np.float32)
    y = acc.T + bo[None, :]
    return np.ascontiguousarray(y.reshape(B, T, D), dtype=np.float32)


def kernel(x, Wq, bq, Wk, bk, Wv, bv, Wo, bo, _trace=False, _debug=False):
    x = np.asarray(x, dtype=np.float32)
    Wq, Wk, Wv, Wo = (np.asarray(w, dtype=np.float32) for w in (Wq, Wk, Wv, Wo))
    bq, bk, bv, bo = (np.asarray(b_, dtype=np.float32) for b_ in (bq, bk, bv, bo))

    with_bias = bool(np.any(bq != 0) or np.any(bk != 0) or np.any(bv != 0))
    nc = _get_nc(with_bias, _debug)
    in_maps = _make_in_maps(x, Wq, bq, Wk, bk, Wv, bv, Wo, with_bias)
    res = run_bass_kernel_spmd(
        nc, in_maps, core_ids=list(range(NCORES)), trace=_trace
    )
    y = _gather(res, bo)
    if _trace or _debug:
        return y, res
    return y
